# revision 5
# baseline (speedup 1.0000x reference)
"""Trainium2 Bass kernel for out = x * exclusive_cumsum(x, axis=time).

Input x: [B=8, T=4096, D=1024] f32. Pure data parallel: batch element b -> core b.

Per-core algorithm (x_c: [T, D], partition axis = time), flat block pipeline:
  - T is split into 32 blocks of 127 output rows + one final 32-row block.
    Each block LOADS 128 rows starting one row early: partition p holds
    x row r0-1+p, so every engine AP sits at base partition 0 (the hardware
    requires quadrant-aligned engine APs; DMAs do not care).
  - Loads are SWDGE cast-DMAs (gpsimd): HBM f32 -> SBUF f16, so no on-chip
    cast pass exists and all 33 xa tiles (~8.5 MB) stay resident, letting
    every load be queued up-front on the Q0 ring while stores stream on the
    sync HWDGE ring; the SDMA engines round-robin the two rings so HBM runs
    read+write concurrently (~420 GB/s combined vs ~350 single-direction).
  - One [128,128] f16 weight WT (WT[k,p] = (k>=1)&((p==0)|(k<p))) computes in
    a single pass: partition p>=1 = exclusive prefix of block row p-1, and
    partition 0 = block column-total (row k=0 is all zero, so the overlap row
    in rhs partition 0 never contributes).
  - A k=1 matmul with an all-ones [1,128] lhsT adds the running carry row to
    every partition, after which ps[0] = carry + colsum = the next carry.
    The carry crosses PSUM->SBUF via per-chunk ACT copies (the only engine
    with PSUM read access that is otherwise idle); DVE does just one
    [128,1024] multiply per block (partition 0 is a throwaway lane). The
    store reads SBUF partitions 1..127. PE/DVE/ACT all sit at ~40-60% so
    the kernel tracks the HBM roofline.
"""

import sys

sys.path.insert(0, "/opt/trn_rl_repo")

import numpy as np

B, T, D = 8, 4096, 1024
BLK = 127            # output rows per full block (loads are 128 rows)
NFULL = T // BLK     # 32
REM = T - NFULL * BLK  # 32
NCH = 2
CH = D // NCH        # 512, one PSUM bank in f32

_CACHE = {}


def _weights():
    # Row k=0 is all zeros (partition 0 of the rhs is the previous block's
    # last row, present only to keep APs base-0 aligned). Column p=0 is ones
    # for k>=1 -> partition 0 = block column-total. Column p>=1: 1 iff
    # 1 <= k < p -> partition p = exclusive prefix for block row p-1.
    k = np.arange(128)[:, None]
    p = np.arange(128)[None, :]
    wt = ((k >= 1) & ((p == 0) | (k < p))).astype(np.float16)
    on = np.ones((1, 128), dtype=np.float16)
    return wt, on


def _blocks():
    blocks = [(i * BLK, BLK) for i in range(NFULL)]
    if REM:
        blocks.append((NFULL * BLK, REM))
    return blocks


def build_nc(num_devices=B):
    """Build the Bass module for one core's [T, D] shard."""
    import concourse.bass as bass
    import concourse.mybir as mybir
    import concourse.tile as tile
    from concourse import bacc

    f32 = mybir.dt.float32
    f16 = mybir.dt.float16

    nc = bacc.Bacc("TRN2", target_bir_lowering=False, debug=False,
                   num_devices=num_devices)
    x = nc.dram_tensor("x", [T, D], f32, kind="ExternalInput").ap()
    wtd = nc.dram_tensor("wt", [128, 128], f16, kind="ExternalInput").ap()
    ond = nc.dram_tensor("ones", [1, 128], f16, kind="ExternalInput").ap()
    out = nc.dram_tensor("out", [T, D], f32, kind="ExternalOutput").ap()

    blocks = _blocks()
    nb = len(blocks)

    with tile.TileContext(nc) as tc:
        with (
            tc.tile_pool(name="wpool", bufs=1) as wpool,
            tc.tile_pool(name="xpool", bufs=nb) as xpool,
            tc.tile_pool(name="rpool", bufs=3) as rpool,
            tc.tile_pool(name="opool", bufs=6) as opool,
            tc.tile_pool(name="ppool", bufs=3,
                         space=bass.MemorySpace.PSUM) as ppool,
        ):
            wt = wpool.tile([128, 128], f16, tag="wt")
            nc.sync.dma_start(wt[:], wtd[:])
            on = wpool.tile([1, 128], f16, tag="on")
            nc.sync.dma_start(on[:], ond[:])

            # All loads issued up-front: every xa tile has its own buffer, so
            # the Pool sequencer streams the emissions with no waits and the
            # load ring always has work for the SDMA round-robin.
            xas = []
            for i, (r0, rows) in enumerate(blocks):
                xa = xpool.tile([128, D], f16, tag="xa", name=f"xa{i}")
                if i == 0:
                    # No row -1: fill partition 0 with row 0 (value unused by
                    # the matmul; keeps the lane initialized for the mul).
                    nc.gpsimd.dma_start(xa[0:1, :], x[0:1, :])
                    nc.gpsimd.dma_start(xa[1:rows + 1, :], x[0:rows, :])
                else:
                    nc.gpsimd.dma_start(xa[0:rows + 1, :],
                                        x[r0 - 1:r0 + rows, :])
                xas.append(xa)

            r_prev = None
            for i, (r0, rows) in enumerate(blocks):
                # partition 0 = total/carry, partitions 1..rows = prefixes
                npart = rows + 1
                ps = ppool.tile([128, D], f32, tag="ps", name=f"ps{i}")
                for j in range(NCH):
                    jc = slice(j * CH, (j + 1) * CH)
                    nc.tensor.matmul(
                        ps[0:npart, jc], wt[0:npart, 0:npart],
                        xas[i][0:npart, jc],
                        start=True, stop=(i == 0))
                if i > 0:
                    for j in range(NCH):
                        jc = slice(j * CH, (j + 1) * CH)
                        nc.tensor.matmul(
                            ps[0:npart, jc], on[0:1, 0:npart],
                            r_prev[0:1, jc],
                            start=False, stop=True)
                if i < nb - 1:
                    # Next carry = ps[0] = running carry + block colsum.
                    rn = rpool.tile([1, D], f16, tag="r", name=f"r{i + 1}")
                    for j in range(NCH):
                        jc = slice(j * CH, (j + 1) * CH)
                        nc.scalar.copy(rn[0:1, jc], ps[0:1, jc])
                else:
                    rn = None
                ot = opool.tile([128, D], f32, tag="ot", name=f"ot{i}")
                nc.vector.tensor_mul(ot[0:npart, :], xas[i][0:npart, :],
                                     ps[0:npart, :])
                nc.sync.dma_start(out[r0:r0 + rows, :], ot[1:rows + 1, :])
                r_prev = rn

    nc.compile()
    return nc


def _in_maps(x):
    wt, on = _weights()
    return [
        {"x": np.ascontiguousarray(x[c]), "wt": wt, "ones": on}
        for c in range(B)
    ]


def kernel(x: np.ndarray) -> np.ndarray:
    from concourse.bass_utils import run_bass_kernel_spmd

    x = np.asarray(x, dtype=np.float32)
    assert x.shape == (B, T, D)
    key = "full"
    if key not in _CACHE:
        _CACHE[key] = build_nc()
    nc = _CACHE[key]

    res = run_bass_kernel_spmd(nc, _in_maps(x), core_ids=list(range(B)))
    return np.stack([res.results[c]["out"] for c in range(B)], axis=0)


# revision 7
# speedup vs baseline: 4.3204x; 4.3204x over previous
"""Trainium2 Bass kernel for out = x * exclusive_cumsum(x, axis=time).

Input x: [B=8, T=4096, D=1024] f32. Pure data parallel: batch element b -> core b.

Per-core algorithm (x_c: [T, D], partition axis = time), flat block pipeline:
  - T is split into 42 blocks of 96 rows + one final 64-row block. Every DMA
    (load and store) is a full base-partition-0 transfer; misaligned bases
    serialize DMA onto one SDMA engine (~26 GB/s measured), so this matters.
  - Loads are SWDGE cast-DMAs (gpsimd): HBM f32 -> SBUF f16, so no on-chip
    cast pass exists and all 43 xa tiles (~8.5 MB) stay resident, letting
    every load be queued up-front on the Q0 ring while stores stream on the
    sync HWDGE ring; the SDMA engines round-robin the two rings so HBM runs
    read+write concurrently (~420 GB/s combined vs ~350 single-direction).
  - A [96,96] strict-upper f16 weight computes the within-block exclusive
    prefixes in one PE pass per 512-chunk. The running carry is added to all
    partitions by two small accumulate matmuls whose operands sit at
    partition base 64 (matmul operand bases must be 0/32/64): a k=1 ones row
    times the carry row rn, plus a k=32 ones block times xa_i[64:96] (the
    rows between the aligned carry anchor and the block end). After those,
    ps_i[64] = prefix(row 96i+64), which an ACT per-chunk copy (base 64,
    no partition shift; ACT is the only otherwise-idle engine with PSUM
    access) turns into the next block's rn. DVE does one [96,1024] multiply
    per block. All engines stay at ~45-75% of the ~80us HBM roofline.
"""

import sys

sys.path.insert(0, "/opt/trn_rl_repo")

import numpy as np

B, T, D = 8, 4096, 1024
BLK = 96             # rows per full block
ANCH = 64            # carry anchor partition (must be 0/32/64 for matmul)
NCH = 2
CH = D // NCH        # 512, one PSUM bank in f32

_CACHE = {}


def _weights():
    # Strict upper triangular: wt[k,p] = 1 iff k < p. Partition p of the
    # matmul output = exclusive prefix of block row p.
    wt = np.triu(np.ones((BLK, BLK), dtype=np.float16), 1)
    # 32 all-ones rows, loaded at partitions 64..95 of the on tile: row 64
    # serves the k=1 carry matmul, rows 64..95 the k=32 tail-colsum matmul.
    on = np.ones((32, 128), dtype=np.float16)
    return wt, on


def _blocks():
    blocks = []
    f = 0
    while f + BLK <= T:
        blocks.append((f, BLK))
        f += BLK
    if f < T:
        blocks.append((f, T - f))
    return blocks


def build_nc(num_devices=B):
    """Build the Bass module for one core's [T, D] shard."""
    import concourse.bass as bass
    import concourse.mybir as mybir
    import concourse.tile as tile
    from concourse import bacc

    f32 = mybir.dt.float32
    f16 = mybir.dt.float16

    nc = bacc.Bacc("TRN2", target_bir_lowering=False, debug=False,
                   num_devices=num_devices)
    x = nc.dram_tensor("x", [T, D], f32, kind="ExternalInput").ap()
    wtd = nc.dram_tensor("wt", [BLK, BLK], f16, kind="ExternalInput").ap()
    ond = nc.dram_tensor("ones", [32, 128], f16, kind="ExternalInput").ap()
    out = nc.dram_tensor("out", [T, D], f32, kind="ExternalOutput").ap()

    blocks = _blocks()
    nb = len(blocks)

    with tile.TileContext(nc) as tc:
        with (
            tc.tile_pool(name="wpool", bufs=1) as wpool,
            tc.tile_pool(name="xpool", bufs=nb) as xpool,
            tc.tile_pool(name="rpool", bufs=3) as rpool,
            tc.tile_pool(name="opool", bufs=6) as opool,
            tc.tile_pool(name="ppool", bufs=3,
                         space=bass.MemorySpace.PSUM) as ppool,
        ):
            wt = wpool.tile([BLK, BLK], f16, tag="wt")
            nc.sync.dma_start(wt[:], wtd[:])
            on = wpool.tile([BLK, 128], f16, tag="on")
            nc.sync.dma_start(on[ANCH:ANCH + 32, :], ond[:])

            # All loads issued up-front: every xa tile has its own buffer, so
            # the Pool sequencer streams the emissions with no waits and the
            # load ring always has work for the SDMA round-robin.
            xas = []
            for i, (f0, rows) in enumerate(blocks):
                xa = xpool.tile([BLK, D], f16, tag="xa", name=f"xa{i}")
                nc.gpsimd.dma_start(xa[0:rows, :], x[f0:f0 + rows, :])
                xas.append(xa)

            r_prev = None
            for i, (f0, rows) in enumerate(blocks):
                ps = ppool.tile([128, D], f32, tag="ps", name=f"ps{i}")
                for j in range(NCH):
                    jc = slice(j * CH, (j + 1) * CH)
                    nc.tensor.matmul(
                        ps[0:rows, jc], wt[0:rows, 0:rows],
                        xas[i][0:rows, jc],
                        start=True, stop=(i == 0))
                if i > 0:
                    for j in range(NCH):
                        jc = slice(j * CH, (j + 1) * CH)
                        # carry = rn (prefix at the previous block's anchor)
                        nc.tensor.matmul(
                            ps[0:rows, jc], on[ANCH:ANCH + 1, 0:rows],
                            r_prev[ANCH:ANCH + 1, jc],
                            start=False, stop=False)
                        # ... plus the previous block's rows past the anchor
                        nc.tensor.matmul(
                            ps[0:rows, jc], on[ANCH:ANCH + 32, 0:rows],
                            xas[i - 1][ANCH:ANCH + 32, jc],
                            start=False, stop=True)
                if i < nb - 1:
                    # Anchor for the next carry: ps[64] = prefix(f0 + 64).
                    rn = rpool.tile([ANCH + 1, D], f16, tag="r",
                                    name=f"r{i + 1}")
                    for j in range(NCH):
                        jc = slice(j * CH, (j + 1) * CH)
                        nc.scalar.copy(rn[ANCH:ANCH + 1, jc],
                                       ps[ANCH:ANCH + 1, jc])
                else:
                    rn = None
                ot = opool.tile([BLK, D], f32, tag="ot", name=f"ot{i}")
                nc.vector.tensor_mul(ot[0:rows, :], xas[i][0:rows, :],
                                     ps[0:rows, :])
                nc.sync.dma_start(out[f0:f0 + rows, :], ot[0:rows, :])
                r_prev = rn

    nc.compile()
    return nc


def _in_maps(x):
    wt, on = _weights()
    return [
        {"x": np.ascontiguousarray(x[c]), "wt": wt, "ones": on}
        for c in range(B)
    ]


def kernel(x: np.ndarray) -> np.ndarray:
    from concourse.bass_utils import run_bass_kernel_spmd

    x = np.asarray(x, dtype=np.float32)
    assert x.shape == (B, T, D)
    key = "full"
    if key not in _CACHE:
        _CACHE[key] = build_nc()
    nc = _CACHE[key]

    res = run_bass_kernel_spmd(nc, _in_maps(x), core_ids=list(range(B)))
    return np.stack([res.results[c]["out"] for c in range(B)], axis=0)


# revision 8
# speedup vs baseline: 6.9725x; 1.6138x over previous
"""Trainium2 Bass kernel for out = x * exclusive_cumsum(x, axis=time).

Input x: [B=8, T=4096, D=1024] f32. Pure data parallel: batch element b -> core b.

Per-core algorithm (x_c: [T, D], partition axis = time), flat block pipeline:
  - T is split into 42 blocks of 96 rows + one final 64-row block. Every bulk
    DMA (load and store) is a base-partition-0 transfer; misaligned bases
    serialize DMA onto one SDMA engine (~26 GB/s measured), so this matters.
  - Loads are SWDGE cast-DMAs (gpsimd): HBM f32 -> SBUF f16, so no on-chip
    cast pass exists and all 43 xa tiles (~8.5 MB) stay resident, letting
    every load be queued up-front on the Q0 ring while stores stream on the
    sync HWDGE ring; the SDMA engines round-robin the two rings so HBM runs
    read+write concurrently (~420 GB/s combined vs ~350 single-direction).
  - ONE matmul per 512-chunk per block does all the math (PE instruction
    overhead is ~0.6us regardless of k, so matmul COUNT is what matters):
    xa tiles are [97, D] with partitions 0..95 = block rows and partition
    96 = the incoming carry row; lhsT is strict-upper triu(97,97) with row
    96 overwritten to all-ones. Then ps[p<96] = carry + exclusive prefix of
    row p, and ps[96] = carry + colsum = the NEXT block's carry, which a
    per-chunk ACT copy (base 96 -> 96, no partition shift; ACT is the only
    otherwise-idle engine with PSUM access) writes into the next xa tile.
  - DVE does one [96,1024] multiply per block; stores are [96, D] f32 on the
    sync ring. The serial carry chain is mm -> ACT copy -> mm per chunk,
    ~1.5us per 1.83us block period, so the bus stays the limiter.
"""

import sys

sys.path.insert(0, "/opt/trn_rl_repo")

import numpy as np

B, T, D = 8, 4096, 1024
BLK = 96             # rows per full block
CAR = 96             # carry-row partition inside xa tiles
NCH = 2
CH = D // NCH        # 512, one PSUM bank in f32

_CACHE = {}


def _weights():
    # wt[k,p] = 1 iff k < p (strict upper: partition p = exclusive prefix of
    # block row p, column 96 = colsum); row 96 = all ones (adds the carry row
    # living at rhs partition 96 to every output partition).
    wt = np.triu(np.ones((97, 97), dtype=np.float16), 1)
    wt[96, :] = 1.0
    return wt


def _blocks():
    blocks = []
    f = 0
    while f + BLK <= T:
        blocks.append((f, BLK))
        f += BLK
    if f < T:
        blocks.append((f, T - f))
    return blocks


def build_nc(num_devices=B):
    """Build the Bass module for one core's [T, D] shard."""
    import concourse.bass as bass
    import concourse.mybir as mybir
    import concourse.tile as tile
    from concourse import bacc

    f32 = mybir.dt.float32
    f16 = mybir.dt.float16

    nc = bacc.Bacc("TRN2", target_bir_lowering=False, debug=False,
                   num_devices=num_devices)
    x = nc.dram_tensor("x", [T, D], f32, kind="ExternalInput").ap()
    wtd = nc.dram_tensor("wt", [97, 97], f16, kind="ExternalInput").ap()
    out = nc.dram_tensor("out", [T, D], f32, kind="ExternalOutput").ap()

    blocks = _blocks()
    nb = len(blocks)

    with tile.TileContext(nc) as tc:
        with (
            tc.tile_pool(name="wpool", bufs=1) as wpool,
            tc.tile_pool(name="xpool", bufs=nb) as xpool,
            tc.tile_pool(name="opool", bufs=6) as opool,
            tc.tile_pool(name="ppool", bufs=4,
                         space=bass.MemorySpace.PSUM) as ppool,
        ):
            wt = wpool.tile([97, 97], f16, tag="wt")
            nc.sync.dma_start(wt[:], wtd[:])

            # All loads issued up-front: every xa tile has its own buffer, so
            # the Pool sequencer streams the emissions with no waits and the
            # load ring always has work for the SDMA round-robin.
            xas = []
            for i, (f0, rows) in enumerate(blocks):
                xa = xpool.tile([97, D], f16, tag="xa", name=f"xa{i}")
                nc.gpsimd.dma_start(xa[0:rows, :], x[f0:f0 + rows, :])
                if rows < BLK:
                    # Final short block: k runs to 96, so zero the unloaded
                    # partitions their (zero-weighted) lanes would otherwise
                    # stream garbage through the PE.
                    nc.vector.memset(xa[rows:BLK, :], 0.0)
                xas.append(xa)

            for i, (f0, rows) in enumerate(blocks):
                last = i == nb - 1
                # Block 0 has no carry: restrict k to the data rows.
                klo = rows if i == 0 else 97
                npart = rows + (0 if last else 1)
                ps = ppool.tile([128, D], f32, tag="ps", name=f"ps{i}")
                for j in range(NCH):
                    jc = slice(j * CH, (j + 1) * CH)
                    nc.tensor.matmul(
                        ps[0:npart, jc], wt[0:klo, 0:npart],
                        xas[i][0:klo, jc],
                        start=True, stop=True)
                    if not last:
                        # ps[96] = carry + colsum = next block's carry row.
                        nc.scalar.copy(xas[i + 1][CAR:CAR + 1, jc],
                                       ps[CAR:CAR + 1, jc])
                ot = opool.tile([BLK, D], f32, tag="ot", name=f"ot{i}")
                nc.vector.tensor_mul(ot[0:rows, :], xas[i][0:rows, :],
                                     ps[0:rows, :])
                nc.sync.dma_start(out[f0:f0 + rows, :], ot[0:rows, :])

    nc.compile()
    return nc


def _in_maps(x):
    wt = _weights()
    return [
        {"x": np.ascontiguousarray(x[c]), "wt": wt}
        for c in range(B)
    ]


def kernel(x: np.ndarray) -> np.ndarray:
    from concourse.bass_utils import run_bass_kernel_spmd

    x = np.asarray(x, dtype=np.float32)
    assert x.shape == (B, T, D)
    key = "full"
    if key not in _CACHE:
        _CACHE[key] = build_nc()
    nc = _CACHE[key]

    res = run_bass_kernel_spmd(nc, _in_maps(x), core_ids=list(range(B)))
    return np.stack([res.results[c]["out"] for c in range(B)], axis=0)
